# revision 11
# baseline (speedup 1.0000x reference)
"""GPTQ-style 4-bit dequantizer on 8 TRN2 NeuronCores.

Strategy (column-parallel per the N-axis sharding hint), v3 "packed-subtract":
  - Shard qweight/scales/qzeros/output along N across 8 cores; g_idx replicated.
  - HOST packs each int32's byte-b nibbles into int16 planes m = v_lo | v_hi<<8.
  - Per core, per plane tile (t, b):
      * psZZ = [oh_even; oh_odd] @ [z; 256*z]   one 64-contraction PE gather of
                                                the packed zero-points (half the
                                                volume of per-j gathers)
      * M    = (m + 16) - psZZ                  one DVE STT (1x, PSUM operand):
               = 256*(v_hi - z_hi) + (v_lo - z_lo + 16), low byte in [1,31]
      * per j in the pair: d = exact shift-extract of (w - z) from M
      * psS  = onehot_j @ s16                   PE gather of bf16 scales
      * s16sb = copy(psS)                       ACT drain -> SBUF bf16
      * out  = d * s16sb (odd j) / (d-16)*s16sb (even j, STT)   DVE 2x
      * strided-row DMA store (rows 8*kpf+j), bf16 (2752 B per row descriptor).
  - Output DRAM tensor is bf16; host upcasts to f32 (rel tolerance allows it).
"""

import numpy as np
from contextlib import ExitStack

import concourse.bacc as bacc
import concourse.bass as bass
import concourse.tile as tile
import concourse.mybir as mybir
from concourse.bass_utils import run_bass_kernel_spmd

K = 4096          # input features (rows of dequantized weight)
N = 11008         # output features
G = 32            # quant groups
PF = 8            # int32 packs 8 nibbles
MAXQ = 0xF
NCORES = 8
NS = N // NCORES        # 1376 columns per core
KP = K // PF            # 512 packed rows
NZS = NS // PF          # 172 packed qzero columns per core
KT = KP // 128          # 4 packed row-tiles
CHUNKS = [(0, 688), (688, 688)]
MMSPLIT = [(0, 512), (512, 176)]

f32 = mybir.dt.float32
bf16 = mybir.dt.bfloat16
i32 = mybir.dt.int32
i16 = mybir.dt.int16
Alu = mybir.AluOpType

_module_cache = {}


def build_module(n_ktiles=KT):
    nc = bacc.Bacc("TRN2", target_bir_lowering=False, debug=False,
                   num_devices=NCORES)
    qb_d = nc.dram_tensor("qbytes", [4 * KP, NS], i16, kind="ExternalInput")
    sc_d = nc.dram_tensor("scales", [G, NS], f32, kind="ExternalInput")
    qz_d = nc.dram_tensor("qzeros", [G, NZS], i32, kind="ExternalInput")
    gi_d = nc.dram_tensor("g_idx", [1, K], i32, kind="ExternalInput")
    out_d = nc.dram_tensor("out", [K, NS], bf16, kind="ExternalOutput")

    with tile.TileContext(nc) as tc, ExitStack() as ctx:
        const = ctx.enter_context(tc.tile_pool(name="const", bufs=1))
        qbp = ctx.enter_context(tc.tile_pool(name="qb", bufs=5))
        mpool = ctx.enter_context(tc.tile_pool(name="M", bufs=3))
        dpool = ctx.enter_context(tc.tile_pool(name="d16", bufs=4))
        outp = ctx.enter_context(tc.tile_pool(name="out", bufs=4))
        psS_p = ctx.enter_context(tc.tile_pool(name="psS", bufs=2, space="PSUM"))
        psZ_p = ctx.enter_context(tc.tile_pool(name="psZZ", bufs=2, space="PSUM"))

        # ---- constants / precompute ----
        scales_sb = const.tile([G, NS], f32)
        nc.sync.dma_start(scales_sb[:], sc_d.ap())
        qz_sb = const.tile([G, NZS], i32)
        nc.sync.dma_start(qz_sb[:], qz_d.ap())

        # g_idx broadcast to 32 partitions
        g_b = const.tile([G, K], i32)
        nc.sync.dma_start(g_b[:], bass.AP(gi_d, 0, [[0, G], [1, K]]))

        iota_col = const.tile([G, 1], f32)
        nc.gpsimd.iota(iota_col[:], [[0, 1]], channel_multiplier=1,
                       allow_small_or_imprecise_dtypes=True)

        # per-j one-hot in (t, j) block order: block u=t*8+j, col p <-> k=1024t+8p+j
        oh_f = const.tile([G, K], f32)
        g5 = g_b[:].rearrange("p (t q e) -> p t e q", t=KT, q=128, e=PF)
        oh_v = oh_f[:].rearrange("p (t e q) -> p t e q", t=KT, e=PF, q=128)
        nc.vector.tensor_scalar(oh_v, g5, iota_col[:], None,
                                op0=Alu.is_equal)
        oh16 = const.tile([G, K], bf16)
        nc.scalar.copy(oh16[:], oh_f[:])

        # stacked (even;odd) one-hot per plane block (t,b): col p <-> k=1024t+8p+2b(+1)
        # rows 0-31: j=2b map; rows 32-63: j=2b+1 map.
        g64 = const.tile([64, K // 2], i32)
        gsplit = g_b[:].rearrange("p (t q e2 d) -> p d t e2 q",
                                  t=KT, q=128, e2=4, d=2)
        g64lo = g64[0:32, :].rearrange("p (t e2 q) -> p t e2 q",
                                       t=KT, e2=4, q=128)
        g64hi = g64[32:64, :].rearrange("p (t e2 q) -> p t e2 q",
                                        t=KT, e2=4, q=128)
        nc.vector.tensor_copy(g64lo, gsplit[:, 0])
        # odd-map rows hold g + 32 so one 0..63 iota compare builds both halves
        nc.vector.tensor_scalar(g64hi, gsplit[:, 1], 32, None, op0=Alu.add)
        iota64 = const.tile([64, 1], f32)
        nc.gpsimd.iota(iota64[:], [[0, 1]], channel_multiplier=1,
                       allow_small_or_imprecise_dtypes=True)
        oh64_f = const.tile([64, K // 2], f32)
        nc.vector.tensor_scalar(oh64_f[:], g64[:], iota64[:], None,
                                op0=Alu.is_equal)
        oh64 = const.tile([64, K // 2], bf16)
        nc.scalar.copy(oh64[:], oh64_f[:])

        # unpack zeros (int32, strided by 8)
        zeros_i = const.tile([G, NS], i32)
        z3 = zeros_i[:].rearrange("p (c e) -> p c e", e=PF)
        for jz in range(PF):
            nc.vector.tensor_scalar(
                z3[:, :, jz], qz_sb[:], 4 * jz, MAXQ,
                op0=Alu.logical_shift_right, op1=Alu.bitwise_and)

        # two-limb packed-zero rhs: rows 0-31 = z, rows 32-63 = 256*z (bf16 exact)
        zz64 = const.tile([64, NS], bf16)
        nc.scalar.copy(zz64[0:32, :], zeros_i[:])
        z256 = const.tile([G, NS], i32)
        nc.vector.tensor_scalar(z256[:], zeros_i[:], 256, None, op0=Alu.mult)
        nc.scalar.copy(zz64[32:64, :], z256[:])

        # bf16 scales for the PE gathers
        s16 = const.tile([G, NS], bf16)
        nc.scalar.copy(s16[:], scales_sb[:])

        # PE warm-up: back-to-back matmuls so HAM ramps before the main loop.
        warm = psS_p.tile([128, 688], f32, tag="psS")
        for _ in range(20):
            nc.tensor.matmul(warm[:, 0:512], oh16[:, 0:128],
                             s16[:, 0:512], start=True, stop=True)

        out4 = out_d.ap().rearrange("(t q e) n -> t q e n", t=KT, q=128, e=PF)

        # ---- main loop ----
        for t in range(n_ktiles):
            for b in range(4):
                qb_t = qbp.tile([128, NS], i16)
                nc.sync.dma_start(
                    qb_t[:],
                    qb_d.ap()[b * KP + t * 128:b * KP + (t + 1) * 128, :])
                oh2 = oh64[:, (t * 4 + b) * 128:(t * 4 + b + 1) * 128]

                # M = (m + 0x1010) - zz : both bytes = (v - z + 16) in [1, 31],
                # so M is positive and each field extracts unsigned.
                Mt = mpool.tile([128, NS], i16)
                for (c0, cw) in CHUNKS:
                    psZZ = psZ_p.tile([128, 688], f32, tag="psZZ")
                    for (m0, mw) in MMSPLIT:
                        nc.tensor.matmul(psZZ[:, m0:m0 + mw], oh2,
                                         zz64[:, c0 + m0:c0 + m0 + mw],
                                         start=True, stop=True)
                    nc.vector.scalar_tensor_tensor(
                        Mt[:, c0:c0 + cw], qb_t[:, c0:c0 + cw], 0x1010,
                        psZZ[:, 0:cw], op0=Alu.add, op1=Alu.subtract)

                for h in range(2):
                    j = 2 * b + h
                    u = t * PF + j
                    oh_u = oh16[:, u * 128:(u + 1) * 128]
                    d16 = dpool.tile([128, NS], i16)
                    if h == 0:
                        # low byte: d + 16 = v_lo - z_lo + 16 in [1, 31]
                        nc.vector.tensor_scalar(
                            d16[:], Mt[:], 0xFF, None, op0=Alu.bitwise_and)
                    else:
                        # high byte: d + 16 = v_hi - z_hi + 16 in [1, 31]
                        nc.vector.tensor_scalar(
                            d16[:], Mt[:], 8, 0xFF,
                            op0=Alu.logical_shift_right, op1=Alu.bitwise_and)
                    ot16 = outp.tile([128, NS], bf16)
                    for (c0, cw) in CHUNKS:
                        psS = psS_p.tile([128, 688], f32, tag="psS")
                        for (m0, mw) in MMSPLIT:
                            nc.tensor.matmul(psS[:, m0:m0 + mw], oh_u,
                                             s16[:, c0 + m0:c0 + m0 + mw],
                                             start=True, stop=True)
                        nc.vector.scalar_tensor_tensor(
                            ot16[:, c0:c0 + cw], d16[:, c0:c0 + cw], 16,
                            psS[:, 0:cw],
                            op0=Alu.subtract, op1=Alu.mult)
                    nc.sync.dma_start(out4[t, :, j, :], ot16[:])

    nc.compile()
    return nc


def get_module():
    if "nc" not in _module_cache:
        _module_cache["nc"] = build_module()
    return _module_cache["nc"]


def make_in_maps(qweight, qzeros, scales, g_idx):
    """Host-side prep: nibble-pair plane split of qweight + per-core sharding."""
    qweight = np.ascontiguousarray(qweight, dtype=np.int32)
    qzeros = np.ascontiguousarray(qzeros, dtype=np.int32)
    scales = np.ascontiguousarray(scales, dtype=np.float32)
    g_idx_2d = np.ascontiguousarray(g_idx, dtype=np.int32).reshape(1, K)

    # nibble-pair planes as int16 (bits 0 and 8): plane b rows [b*KP, (b+1)*KP)
    qbytes = np.concatenate(
        [(((qweight >> (8 * b)) & 0xF)
          | (((qweight >> (8 * b + 4)) & 0xF) << 8)).astype(np.int16)
         for b in range(4)],
        axis=0)

    in_maps = []
    for c in range(NCORES):
        nlo, nhi = c * NS, (c + 1) * NS
        in_maps.append({
            "qbytes": np.ascontiguousarray(qbytes[:, nlo:nhi]),
            "scales": np.ascontiguousarray(scales[:, nlo:nhi]),
            "qzeros": np.ascontiguousarray(qzeros[:, c * NZS:(c + 1) * NZS]),
            "g_idx": g_idx_2d,
        })
    return in_maps


def kernel(qweight, qzeros, scales, g_idx):
    nc = get_module()
    in_maps = make_in_maps(qweight, qzeros, scales, g_idx)
    res = run_bass_kernel_spmd(nc, in_maps, list(range(NCORES))).results
    out = np.concatenate(
        [np.asarray(res[c]["out"]).astype(np.float32) for c in range(NCORES)],
        axis=1)
    return np.ascontiguousarray(out, dtype=np.float32)


# revision 13
# speedup vs baseline: 1.0196x; 1.0196x over previous
"""GPTQ-style 4-bit dequantizer on 8 TRN2 NeuronCores.

Strategy (column-parallel per the N-axis sharding hint), v3 "packed-subtract":
  - Shard qweight/scales/qzeros/output along N across 8 cores; g_idx replicated.
  - HOST packs each int32's byte-b nibbles into int16 planes m = v_lo | v_hi<<8.
  - Per core, per plane tile (t, b):
      * psZZ = [oh_even; oh_odd] @ [z; 256*z]   one 64-contraction PE gather of
                                                the packed zero-points (half the
                                                volume of per-j gathers)
      * M    = (m + 16) - psZZ                  one DVE STT (1x, PSUM operand):
               = 256*(v_hi - z_hi) + (v_lo - z_lo + 16), low byte in [1,31]
      * per j in the pair: d = exact shift-extract of (w - z) from M
      * psS  = onehot_j @ s16                   PE gather of bf16 scales
      * s16sb = copy(psS)                       ACT drain -> SBUF bf16
      * out  = d * s16sb (odd j) / (d-16)*s16sb (even j, STT)   DVE 2x
      * strided-row DMA store (rows 8*kpf+j), bf16 (2752 B per row descriptor).
  - Output DRAM tensor is bf16; host upcasts to f32 (rel tolerance allows it).
"""

import numpy as np
from contextlib import ExitStack

import concourse.bacc as bacc
import concourse.bass as bass
import concourse.tile as tile
import concourse.mybir as mybir
from concourse.bass_utils import run_bass_kernel_spmd

K = 4096          # input features (rows of dequantized weight)
N = 11008         # output features
G = 32            # quant groups
PF = 8            # int32 packs 8 nibbles
MAXQ = 0xF
NCORES = 8
NS = N // NCORES        # 1376 columns per core
KP = K // PF            # 512 packed rows
NZS = NS // PF          # 172 packed qzero columns per core
KT = KP // 128          # 4 packed row-tiles
CHUNKS = [(0, 688), (688, 688)]
MMSPLIT = [(0, 512), (512, 176)]

f32 = mybir.dt.float32
bf16 = mybir.dt.bfloat16
i32 = mybir.dt.int32
i16 = mybir.dt.int16
Alu = mybir.AluOpType

_module_cache = {}


def build_module(n_ktiles=KT):
    nc = bacc.Bacc("TRN2", target_bir_lowering=False, debug=False,
                   num_devices=NCORES)
    qb_d = nc.dram_tensor("qbytes", [4 * KP, NS], i16, kind="ExternalInput")
    sc_d = nc.dram_tensor("scales", [G, NS], f32, kind="ExternalInput")
    qz_d = nc.dram_tensor("qzeros", [G, NZS], i32, kind="ExternalInput")
    gi_d = nc.dram_tensor("g_idx", [1, K], i32, kind="ExternalInput")
    out_d = nc.dram_tensor("out", [K, NS], bf16, kind="ExternalOutput")

    with tile.TileContext(nc) as tc, ExitStack() as ctx:
        const = ctx.enter_context(tc.tile_pool(name="const", bufs=1))
        qbp = ctx.enter_context(tc.tile_pool(name="qb", bufs=4))
        mpool = ctx.enter_context(tc.tile_pool(name="M", bufs=3))
        dpool = ctx.enter_context(tc.tile_pool(name="d16", bufs=5))
        spool = ctx.enter_context(tc.tile_pool(name="s16sb", bufs=6))
        outp = ctx.enter_context(tc.tile_pool(name="out", bufs=6))
        psS_p = ctx.enter_context(tc.tile_pool(name="psS", bufs=2, space="PSUM"))
        psZ_p = ctx.enter_context(tc.tile_pool(name="psZZ", bufs=2, space="PSUM"))

        # ---- constants / precompute ----
        scales_sb = const.tile([G, NS], f32)
        nc.sync.dma_start(scales_sb[:], sc_d.ap())
        qz_sb = const.tile([G, NZS], i32)
        nc.sync.dma_start(qz_sb[:], qz_d.ap())

        # g_idx broadcast to 32 partitions
        g_b = const.tile([G, K], i32)
        nc.sync.dma_start(g_b[:], bass.AP(gi_d, 0, [[0, G], [1, K]]))

        iota_col = const.tile([G, 1], f32)
        nc.gpsimd.iota(iota_col[:], [[0, 1]], channel_multiplier=1,
                       allow_small_or_imprecise_dtypes=True)

        # per-j one-hot in (t, j) block order: block u=t*8+j, col p <-> k=1024t+8p+j
        oh_f = const.tile([G, K], f32)
        g5 = g_b[:].rearrange("p (t q e) -> p t e q", t=KT, q=128, e=PF)
        oh_v = oh_f[:].rearrange("p (t e q) -> p t e q", t=KT, e=PF, q=128)
        nc.vector.tensor_scalar(oh_v, g5, iota_col[:], None,
                                op0=Alu.is_equal)
        oh16 = const.tile([G, K], bf16)
        nc.scalar.copy(oh16[:], oh_f[:])

        # stacked (even;odd) one-hot per plane block (t,b): col p <-> k=1024t+8p+2b(+1)
        # rows 0-31: j=2b map; rows 32-63: j=2b+1 map.
        g64 = const.tile([64, K // 2], i32)
        gsplit = g_b[:].rearrange("p (t q e2 d) -> p d t e2 q",
                                  t=KT, q=128, e2=4, d=2)
        g64lo = g64[0:32, :].rearrange("p (t e2 q) -> p t e2 q",
                                       t=KT, e2=4, q=128)
        g64hi = g64[32:64, :].rearrange("p (t e2 q) -> p t e2 q",
                                        t=KT, e2=4, q=128)
        nc.vector.tensor_copy(g64lo, gsplit[:, 0])
        # odd-map rows hold g + 32 so one 0..63 iota compare builds both halves
        nc.vector.tensor_scalar(g64hi, gsplit[:, 1], 32, None, op0=Alu.add)
        iota64 = const.tile([64, 1], f32)
        nc.gpsimd.iota(iota64[:], [[0, 1]], channel_multiplier=1,
                       allow_small_or_imprecise_dtypes=True)
        oh64_f = const.tile([64, K // 2], f32)
        nc.vector.tensor_scalar(oh64_f[:], g64[:], iota64[:], None,
                                op0=Alu.is_equal)
        oh64 = const.tile([64, K // 2], bf16)
        nc.scalar.copy(oh64[:], oh64_f[:])

        # unpack zeros (int32, strided by 8)
        zeros_i = const.tile([G, NS], i32)
        z3 = zeros_i[:].rearrange("p (c e) -> p c e", e=PF)
        for jz in range(PF):
            nc.vector.tensor_scalar(
                z3[:, :, jz], qz_sb[:], 4 * jz, MAXQ,
                op0=Alu.logical_shift_right, op1=Alu.bitwise_and)

        # two-limb packed-zero rhs: rows 0-31 = z, rows 32-63 = 256*z (bf16 exact)
        zz64 = const.tile([64, NS], bf16)
        nc.scalar.copy(zz64[0:32, :], zeros_i[:])
        z256 = const.tile([G, NS], i32)
        nc.vector.tensor_scalar(z256[:], zeros_i[:], 256, None, op0=Alu.mult)
        nc.scalar.copy(zz64[32:64, :], z256[:])

        # bf16 scales for the PE gathers
        s16 = const.tile([G, NS], bf16)
        nc.scalar.copy(s16[:], scales_sb[:])

        # PE warm-up: back-to-back matmuls so HAM ramps before the main loop.
        warm = psS_p.tile([128, 688], f32, tag="psS")
        for _ in range(20):
            nc.tensor.matmul(warm[:, 0:512], oh16[:, 0:128],
                             s16[:, 0:512], start=True, stop=True)

        out4 = out_d.ap().rearrange("(t q e) n -> t q e n", t=KT, q=128, e=PF)

        # ---- main loop ----
        for t in range(n_ktiles):
            for b in range(4):
                qb_t = qbp.tile([128, NS], i16)
                nc.sync.dma_start(
                    qb_t[:],
                    qb_d.ap()[b * KP + t * 128:b * KP + (t + 1) * 128, :])
                oh2 = oh64[:, (t * 4 + b) * 128:(t * 4 + b + 1) * 128]

                # M = (m + 0x1010) - zz : both bytes = (v - z + 16) in [1, 31],
                # so M is positive and each field extracts unsigned.
                Mt = mpool.tile([128, NS], i16)
                for (c0, cw) in CHUNKS:
                    psZZ = psZ_p.tile([128, 688], f32, tag="psZZ")
                    for (m0, mw) in MMSPLIT:
                        nc.tensor.matmul(psZZ[:, m0:m0 + mw], oh2,
                                         zz64[:, c0 + m0:c0 + m0 + mw],
                                         start=True, stop=True)
                    nc.vector.scalar_tensor_tensor(
                        Mt[:, c0:c0 + cw], qb_t[:, c0:c0 + cw], 0x1010,
                        psZZ[:, 0:cw], op0=Alu.add, op1=Alu.subtract)

                for h in range(2):
                    j = 2 * b + h
                    u = t * PF + j
                    oh_u = oh16[:, u * 128:(u + 1) * 128]
                    d16 = dpool.tile([128, NS], i16)
                    if h == 0:
                        # low byte: d + 16 = v_lo - z_lo + 16 in [1, 31]
                        nc.vector.tensor_scalar(
                            d16[:], Mt[:], 0xFF, None, op0=Alu.bitwise_and)
                    else:
                        # high byte: d + 16 = v_hi - z_hi + 16 in [1, 31]
                        nc.vector.tensor_scalar(
                            d16[:], Mt[:], 8, 0xFF,
                            op0=Alu.logical_shift_right, op1=Alu.bitwise_and)
                    ot16 = outp.tile([128, NS], bf16)
                    for (c0, cw) in CHUNKS:
                        psS = psS_p.tile([128, 688], f32, tag="psS")
                        for (m0, mw) in MMSPLIT:
                            nc.tensor.matmul(psS[:, m0:m0 + mw], oh_u,
                                             s16[:, c0 + m0:c0 + m0 + mw],
                                             start=True, stop=True)
                        s16sb = spool.tile([128, 688], bf16)
                        nc.scalar.copy(s16sb[:, 0:cw], psS[:, 0:cw])
                        nc.vector.scalar_tensor_tensor(
                            ot16[:, c0:c0 + cw], d16[:, c0:c0 + cw], 16,
                            s16sb[:, 0:cw],
                            op0=Alu.subtract, op1=Alu.mult)
                    nc.sync.dma_start(out4[t, :, j, :], ot16[:])

    nc.compile()
    return nc


def get_module():
    if "nc" not in _module_cache:
        _module_cache["nc"] = build_module()
    return _module_cache["nc"]


def make_in_maps(qweight, qzeros, scales, g_idx):
    """Host-side prep: nibble-pair plane split of qweight + per-core sharding."""
    qweight = np.ascontiguousarray(qweight, dtype=np.int32)
    qzeros = np.ascontiguousarray(qzeros, dtype=np.int32)
    scales = np.ascontiguousarray(scales, dtype=np.float32)
    g_idx_2d = np.ascontiguousarray(g_idx, dtype=np.int32).reshape(1, K)

    # nibble-pair planes as int16 (bits 0 and 8): plane b rows [b*KP, (b+1)*KP)
    qbytes = np.concatenate(
        [(((qweight >> (8 * b)) & 0xF)
          | (((qweight >> (8 * b + 4)) & 0xF) << 8)).astype(np.int16)
         for b in range(4)],
        axis=0)

    in_maps = []
    for c in range(NCORES):
        nlo, nhi = c * NS, (c + 1) * NS
        in_maps.append({
            "qbytes": np.ascontiguousarray(qbytes[:, nlo:nhi]),
            "scales": np.ascontiguousarray(scales[:, nlo:nhi]),
            "qzeros": np.ascontiguousarray(qzeros[:, c * NZS:(c + 1) * NZS]),
            "g_idx": g_idx_2d,
        })
    return in_maps


def kernel(qweight, qzeros, scales, g_idx):
    nc = get_module()
    in_maps = make_in_maps(qweight, qzeros, scales, g_idx)
    res = run_bass_kernel_spmd(nc, in_maps, list(range(NCORES))).results
    out = np.concatenate(
        [np.asarray(res[c]["out"]).astype(np.float32) for c in range(NCORES)],
        axis=1)
    return np.ascontiguousarray(out, dtype=np.float32)
